# revision 6
# baseline (speedup 1.0000x reference)
import numpy as np

_CACHE = {}

N_CORES = 8
TOK = 16384
TOK_PER = TOK // N_CORES  # 2048 tokens per core
DIM = 2048
NE = 64
TOPK = 8
KC = 128            # contraction chunk (partition dim)
NK = DIM // KC      # 16 chunks
NT = 512            # token tile = one f32 PSUM bank
NJ = TOK_PER // NT  # 4 token tiles
N_WARM = 7          # PE warm-up matmuls before real data arrives
N_FILL = 2          # filler matmuls per chunk to keep HAM unthrottled


def _build():
    import concourse.bass as bass
    import concourse.tile as tile
    from concourse import bacc, mybir

    nc = bacc.Bacc(
        "TRN2",
        target_bir_lowering=False,
        debug=False,
        enable_asserts=False,
        num_devices=N_CORES,
    )
    # fp16 hi/lo split of x^T, prepared on host: x = xh + xl exactly to ~22
    # mantissa bits; same for W packed as [Wh | Wl] along 128 stationary cols.
    xh = nc.dram_tensor("xh", (DIM, TOK_PER), mybir.dt.float16, kind="ExternalInput").ap()
    xl = nc.dram_tensor("xl", (DIM, TOK_PER), mybir.dt.float16, kind="ExternalInput").ap()
    # wc: partition-major packed: wc[p, k*128 + c] = Wcat[k*128 + p, c]
    wc = nc.dram_tensor("wc", (KC, NK * 2 * NE), mybir.dt.float16, kind="ExternalInput").ap()
    out = nc.dram_tensor("o", (NE, TOK_PER), mybir.dt.float32, kind="ExternalOutput").ap()

    f16 = mybir.dt.float16
    f32 = mybir.dt.float32

    with tile.TileContext(nc) as tc:
        with (
            tc.tile_pool(name="warm", bufs=1) as warmpool,
            tc.tile_pool(name="wpool", bufs=1) as wpool,
            tc.tile_pool(name="xhpool", bufs=NK) as xhpool,
            tc.tile_pool(name="xlpool", bufs=NK) as xlpool,
            tc.tile_pool(name="lpool", bufs=NJ) as lpool,
            tc.tile_pool(name="opool", bufs=NJ) as opool,
            tc.tile_pool(name="psum", bufs=1, space=bass.MemorySpace.PSUM) as psum,
            tc.tile_pool(name="psumw", bufs=1, space=bass.MemorySpace.PSUM) as psumw,
        ):
            # --- PE warm-up: keep TensorE busy from kernel start so HAM
            # unthrottles to 2.4GHz before the real matmuls begin.
            wsrc = warmpool.tile([KC, 2 * NE], f16)
            wmov = warmpool.tile([KC, NT], f16)
            nc.gpsimd.memset(wsrc[:], 0.0)
            nc.gpsimd.memset(wmov[:], 0.0)
            wacc = psumw.tile([2 * NE, NT], f32)
            for _ in range(N_WARM):
                nc.tensor.matmul(wacc[:], wsrc[:], wmov[:], start=True, stop=True)

            # --- input DMAs: W on the scalar HWDGE ring (parallel with x),
            # x hi/lo chunks interleaved on the sync ring.
            wt = wpool.tile([KC, NK * 2 * NE], f16)
            nc.scalar.dma_start(wt[:], wc[:, :])
            xhts, xlts = [], []
            for k in range(NK):
                xht = xhpool.tile([KC, TOK_PER], f16)
                nc.sync.dma_start(xht[:], xh[k * KC:(k + 1) * KC, :])
                xlt = xlpool.tile([KC, TOK_PER], f16)
                nc.sync.dma_start(xlt[:], xl[k * KC:(k + 1) * KC, :])
                xhts.append(xht)
                xlts.append(xlt)

            # --- matmuls: stationary = [Wh_k | Wl_k] (128 cols); for each k
            # stream hi then lo moving tiles; PSUM rows 0-63 accumulate the
            # Wh product, rows 64-127 the Wl product.
            accs = [
                psum.tile([2 * NE, NT], f32, name=f"acc{j}", tag=f"acc{j}")
                for j in range(NJ)
            ]
            for k in range(NK):
                wk = wt[:, k * 2 * NE:(k + 1) * 2 * NE]
                js = range(NJ) if k < NK - 1 else range(NJ - 1, -1, -1)
                for j in js:
                    nc.tensor.matmul(
                        accs[j][:],
                        wk,
                        xhts[k][:, j * NT:(j + 1) * NT],
                        start=(k == 0),
                        stop=False,
                    )
                    nc.tensor.matmul(
                        accs[j][:],
                        wk,
                        xlts[k][:, j * NT:(j + 1) * NT],
                        start=False,
                        stop=(k == NK - 1),
                    )
                if k < NK - 1:
                    # fillers gated on this chunk's data: absorb the DMA-wait
                    # gap before chunk k+1 so the PE clock stays at 8/8
                    for _ in range(N_FILL):
                        nc.tensor.matmul(
                            wacc[:], wsrc[:], xhts[k][:, 0:NT],
                            start=True, stop=True,
                        )

            # --- fold halves: logits = acc[0:64] + acc[64:128]; DVE may read
            # only one PSUM operand, so stage the lo half through SBUF.
            for j in range(NJ - 1, -1, -1):
                lo = lpool.tile([NE, NT], f32, name=f"lo{j}", tag="lo")
                nc.vector.tensor_copy(lo[:], accs[j][NE:2 * NE, :])
                ot = opool.tile([NE, NT], f32, name=f"ot{j}", tag="ot")
                nc.vector.tensor_tensor(
                    ot[:], accs[j][0:NE, :], lo[:], op=mybir.AluOpType.add
                )
                nc.scalar.dma_start(out[:, j * NT:(j + 1) * NT], ot[:])
    nc.compile()
    return nc


def _prepare_in_maps(x, W):
    x = np.asarray(x, dtype=np.float32)
    W = np.asarray(W, dtype=np.float32)

    # W: transpose to (DIM, NE), fp16 hi/lo split, pack [Wh | Wl] along cols,
    # then partition-major relayout wc[p, k*128 + c] = Wcat[k*128 + p, c]
    WT = np.ascontiguousarray(W.T)                       # (DIM, NE)
    Wh = WT.astype(np.float16)
    Wl = (WT - Wh.astype(np.float32)).astype(np.float16)
    Wcat = np.concatenate([Wh, Wl], axis=1)              # (DIM, 128)
    wc = np.ascontiguousarray(
        Wcat.reshape(NK, KC, 2 * NE).transpose(1, 0, 2).reshape(KC, NK * 2 * NE)
    )

    in_maps = []
    for i in range(N_CORES):
        xsT = x[i * TOK_PER:(i + 1) * TOK_PER].T         # (DIM, TOK_PER) view
        xhi = np.ascontiguousarray(xsT.astype(np.float16))
        xlo = np.ascontiguousarray(
            (xsT - xhi.astype(np.float32)).astype(np.float16)
        )
        in_maps.append({"xh": xhi, "xl": xlo, "wc": wc})
    return in_maps


def kernel(x, W):
    from concourse import bass_utils

    if "nc" not in _CACHE:
        _CACHE["nc"] = _build()
    nc = _CACHE["nc"]

    in_maps = _prepare_in_maps(x, W)
    res = bass_utils.run_bass_kernel_spmd(nc, in_maps, list(range(N_CORES)))
    logits = np.concatenate(
        [np.asarray(r["o"]) for r in res.results], axis=1
    ).T.astype(np.float32)                               # (TOK, NE)

    m = logits.max(axis=-1, keepdims=True)
    e = np.exp(logits - m)
    scores = e / e.sum(axis=-1, keepdims=True)
    idx = np.argsort(-scores, axis=-1, kind="stable")[:, :TOPK].astype(np.int32)
    w = np.take_along_axis(scores, idx, axis=-1).astype(np.float32)
    return w, idx


# revision 7
# speedup vs baseline: 1.0142x; 1.0142x over previous
import numpy as np

_CACHE = {}

N_CORES = 8
TOK = 16384
TOK_PER = TOK // N_CORES  # 2048 tokens per core
DIM = 2048
NE = 64
TOPK = 8
KC = 128            # contraction chunk (partition dim)
NK = DIM // KC      # 16 chunks
NT = 512            # token tile = one f32 PSUM bank
NJ = TOK_PER // NT  # 4 token tiles
N_WARM = 15         # PE warm-up matmuls until first chunk is consumable
N_FILL = 2          # filler matmuls per chunk to keep HAM unthrottled


def _build():
    import concourse.bass as bass
    import concourse.tile as tile
    from concourse import bacc, mybir

    nc = bacc.Bacc(
        "TRN2",
        target_bir_lowering=False,
        debug=False,
        enable_asserts=False,
        num_devices=N_CORES,
    )
    # fp16 hi/lo split of x^T, prepared on host: x = xh + xl exactly to ~22
    # mantissa bits; same for W packed as [Wh | Wl] along 128 stationary cols.
    xh = nc.dram_tensor("xh", (DIM, TOK_PER), mybir.dt.float16, kind="ExternalInput").ap()
    xl = nc.dram_tensor("xl", (DIM, TOK_PER), mybir.dt.float16, kind="ExternalInput").ap()
    # wc: partition-major packed: wc[p, k*128 + c] = Wcat[k*128 + p, c]
    wc = nc.dram_tensor("wc", (KC, NK * 2 * NE), mybir.dt.float16, kind="ExternalInput").ap()
    out = nc.dram_tensor("o", (NE, TOK_PER), mybir.dt.float32, kind="ExternalOutput").ap()

    f16 = mybir.dt.float16
    f32 = mybir.dt.float32

    with tile.TileContext(nc) as tc:
        with (
            tc.tile_pool(name="warm", bufs=1) as warmpool,
            tc.tile_pool(name="wpool", bufs=1) as wpool,
            tc.tile_pool(name="x0pool", bufs=2 * NJ) as x0pool,
            tc.tile_pool(name="xhpool", bufs=NK - 1) as xhpool,
            tc.tile_pool(name="xlpool", bufs=NK - 1) as xlpool,
            tc.tile_pool(name="lpool", bufs=NJ) as lpool,
            tc.tile_pool(name="opool", bufs=NJ) as opool,
            tc.tile_pool(name="psum", bufs=1, space=bass.MemorySpace.PSUM) as psum,
            tc.tile_pool(name="psumw", bufs=1, space=bass.MemorySpace.PSUM) as psumw,
        ):
            # --- PE warm-up: keep TensorE busy from kernel start so HAM
            # unthrottles to 2.4GHz right when the first chunk is consumable.
            wsrc = warmpool.tile([KC, 2 * NE], f16)
            wmov = warmpool.tile([KC, NT], f16)
            nc.gpsimd.memset(wsrc[:], 0.0)
            nc.gpsimd.memset(wmov[:], 0.0)
            wacc = psumw.tile([2 * NE, NT], f32)
            for _ in range(N_WARM):
                nc.tensor.matmul(wacc[:], wsrc[:], wmov[:], start=True, stop=True)
            # ACT warm-up: first activation op pays a table-load cost; pay it
            # here instead of in the output fold.
            awarm = warmpool.tile([KC, 2 * NE], f16)
            nc.scalar.copy(awarm[:], wsrc[:])

            # --- input DMAs, all on the sync HWDGE ring in consumption order.
            # W first (critical path to the first real matmul), then chunk 0
            # split per j-tile (128KB pieces → earliest possible start), then
            # full 512KB hi/lo chunks.
            wt = wpool.tile([KC, NK * 2 * NE], f16)
            nc.sync.dma_start(wt[:], wc[:, :])
            xh0s, xl0s = [], []
            for j in range(NJ):
                xh0 = x0pool.tile([KC, NT], f16, name=f"xh0_{j}", tag=f"xh0_{j}")
                nc.sync.dma_start(xh0[:], xh[0:KC, j * NT:(j + 1) * NT])
                xl0 = x0pool.tile([KC, NT], f16, name=f"xl0_{j}", tag=f"xl0_{j}")
                nc.sync.dma_start(xl0[:], xl[0:KC, j * NT:(j + 1) * NT])
                xh0s.append(xh0)
                xl0s.append(xl0)
            xhts, xlts = [None], [None]
            for k in range(1, NK):
                xht = xhpool.tile([KC, TOK_PER], f16)
                nc.sync.dma_start(xht[:], xh[k * KC:(k + 1) * KC, :])
                xlt = xlpool.tile([KC, TOK_PER], f16)
                nc.sync.dma_start(xlt[:], xl[k * KC:(k + 1) * KC, :])
                xhts.append(xht)
                xlts.append(xlt)

            # --- matmuls: stationary = [Wh_k | Wl_k] (128 cols); for each k
            # stream hi then lo moving tiles; PSUM rows 0-63 accumulate the
            # Wh product, rows 64-127 the Wl product.
            accs = [
                psum.tile([2 * NE, NT], f32, name=f"acc{j}", tag=f"acc{j}")
                for j in range(NJ)
            ]
            for k in range(NK):
                wk = wt[:, k * 2 * NE:(k + 1) * 2 * NE]
                js = range(NJ) if k < NK - 1 else range(NJ - 1, -1, -1)
                for j in js:
                    if k == 0:
                        hi_mov = xh0s[j][:]
                        lo_mov = xl0s[j][:]
                    else:
                        hi_mov = xhts[k][:, j * NT:(j + 1) * NT]
                        lo_mov = xlts[k][:, j * NT:(j + 1) * NT]
                    nc.tensor.matmul(
                        accs[j][:], wk, hi_mov, start=(k == 0), stop=False
                    )
                    nc.tensor.matmul(
                        accs[j][:], wk, lo_mov, start=False, stop=(k == NK - 1)
                    )
                if k < NK - 1:
                    # fillers gated on this chunk's data: absorb the DMA-wait
                    # gap before chunk k+1 so the PE clock stays at 8/8
                    fill_mov = xh0s[0][:] if k == 0 else xhts[k][:, 0:NT]
                    for _ in range(N_FILL):
                        nc.tensor.matmul(
                            wacc[:], wsrc[:], fill_mov, start=True, stop=True
                        )

            # --- fold halves: logits = acc[0:64] + acc[64:128]. DVE may read
            # only one PSUM operand, so stage the lo half through SBUF on the
            # ACT engine (native PSUM reader) while DVE does the adds.
            for j in range(NJ - 1, -1, -1):
                lo = lpool.tile([NE, NT], f32, name=f"lo{j}", tag="lo")
                nc.scalar.copy(lo[:], accs[j][NE:2 * NE, :])
                ot = opool.tile([NE, NT], f32, name=f"ot{j}", tag="ot")
                nc.vector.tensor_tensor(
                    ot[:], accs[j][0:NE, :], lo[:], op=mybir.AluOpType.add
                )
                nc.sync.dma_start(out[:, j * NT:(j + 1) * NT], ot[:])
    nc.compile()
    return nc


def _prepare_in_maps(x, W):
    x = np.asarray(x, dtype=np.float32)
    W = np.asarray(W, dtype=np.float32)

    # W: transpose to (DIM, NE), fp16 hi/lo split, pack [Wh | Wl] along cols,
    # then partition-major relayout wc[p, k*128 + c] = Wcat[k*128 + p, c]
    WT = np.ascontiguousarray(W.T)                       # (DIM, NE)
    Wh = WT.astype(np.float16)
    Wl = (WT - Wh.astype(np.float32)).astype(np.float16)
    Wcat = np.concatenate([Wh, Wl], axis=1)              # (DIM, 128)
    wc = np.ascontiguousarray(
        Wcat.reshape(NK, KC, 2 * NE).transpose(1, 0, 2).reshape(KC, NK * 2 * NE)
    )

    in_maps = []
    for i in range(N_CORES):
        xsT = x[i * TOK_PER:(i + 1) * TOK_PER].T         # (DIM, TOK_PER) view
        xhi = np.ascontiguousarray(xsT.astype(np.float16))
        xlo = np.ascontiguousarray(
            (xsT - xhi.astype(np.float32)).astype(np.float16)
        )
        in_maps.append({"xh": xhi, "xl": xlo, "wc": wc})
    return in_maps


def kernel(x, W):
    from concourse import bass_utils

    if "nc" not in _CACHE:
        _CACHE["nc"] = _build()
    nc = _CACHE["nc"]

    in_maps = _prepare_in_maps(x, W)
    res = bass_utils.run_bass_kernel_spmd(nc, in_maps, list(range(N_CORES)))
    logits = np.concatenate(
        [np.asarray(r["o"]) for r in res.results], axis=1
    ).T.astype(np.float32)                               # (TOK, NE)

    m = logits.max(axis=-1, keepdims=True)
    e = np.exp(logits - m)
    scores = e / e.sum(axis=-1, keepdims=True)
    idx = np.argsort(-scores, axis=-1, kind="stable")[:, :TOPK].astype(np.int32)
    w = np.take_along_axis(scores, idx, axis=-1).astype(np.float32)
    return w, idx
